# revision 40
# baseline (speedup 1.0000x reference)
"""Trainium2 Bass kernel for nn_Attention_11287174054323.

Full attention layer: QKV projections + RoPE + softmax attention + output
projection.  B=2, S=2048, DIM=2048, 16 heads x 128 head_dim, fp32 I/O.

Sharding: tensor-parallel over heads across 8 NeuronCores (2 heads/core).
Each core computes q/k/v projections for its head slice, full attention for
its heads, and a partial output projection (row slice of Wo); the host sums
the 8 partials.

Structure (single merged schedule, one PE stream):
  - Window A: QKV projections (token chunks tcn 0-7, fp16, RoPE fused into
    the Q/K PSUM eviction).  Batch-0 attention only needs tcn 0-3 outputs,
    so its micro-steps (one 128-key score matmul -> exp -> AV accumulate)
    are interleaved into the projection stream for tcn 4-7: the exp work
    lands on the otherwise-idle ACT engine while projections keep the PE
    dense, and projection matmuls fill the exp-latency gaps of attention.
  - Window B: batch-1 attention with the output projection (ph3) finely
    interleaved (per 512-col en-group), plus the batch-0 out-projection.
    PSUM evictions are split into 256-wide halves spread over ACT and DVE.
  - Softmax normalization is deferred past the AV accumulation; the
    denominator is accumulated with DVE adds in two independent
    accumulators, reduced+broadcast with a ones-matmul, inverted with
    reciprocal_approx_fast.
  - PSUM budget (8 banks): qk(2, reused by ph3 in window B) + v(1) +
    s(2) + at(2) + bc(1).
  - PE warm-up matmuls at t=0 start the p-state ramp during the first DMAs.
  - The partial output is staged to fp16 and summed on the host in fp32.
"""

import sys

sys.path.insert(0, "/opt/trn_rl_repo")

import numpy as np

import concourse.tile as tile
import concourse.mybir as mybir
from concourse import bacc
from concourse.bass_utils import run_bass_kernel_spmd

P = 128
B, S, DIM = 2, 2048, 2048
TOK = B * S                     # 4096 tokens
HEADS_PER_CORE = 2
INNER_C = HEADS_PER_CORE * P    # 256 per-core inner dim
KC = DIM // P                   # 16 contraction chunks
TC = 512                        # phase-1 token chunk
NTC = TOK // TC                 # 8
IC = 512                        # attention i-chunk (queries)
NIC = S // IC                   # 4 per (batch, head)
NJC = S // P                    # 16 key chunks per (batch, head)
SCALE = float(P) ** -0.5

F32 = mybir.dt.float32
F16 = mybir.dt.float16
MM = mybir.dt.float16           # matmul dtype for projections (x, wq/wk/wv)
MM_NP = np.float16
AVD = mybir.dt.float16          # dtype of the exp/AV group (e, acc, v, ones)
AVD_NP = np.float16

N_CORES = 8
Mul = mybir.AluOpType.mult
N_WARM = 18                     # PE warm-up matmuls


def _build():
    nc = bacc.Bacc("TRN2", target_bir_lowering=False)

    xT_d = nc.dram_tensor("xT", [NTC, P, KC, TC], MM, kind="ExternalInput")
    wq_d = nc.dram_tensor("wq", [P, KC, INNER_C], MM, kind="ExternalInput")
    wk_d = nc.dram_tensor("wk", [P, KC, INNER_C], MM, kind="ExternalInput")
    wv_d = nc.dram_tensor("wv", [P, KC, INNER_C], MM, kind="ExternalInput")
    wo_d = nc.dram_tensor("wo", [P, 2, DIM], F16, kind="ExternalInput")
    cos_d = nc.dram_tensor("cosT", [P, TOK], F32, kind="ExternalInput")
    sin_d = nc.dram_tensor("sinZ", [P, TOK], F32, kind="ExternalInput")
    ones_d = nc.dram_tensor("ones", [P, P], AVD, kind="ExternalInput")
    o_d = nc.dram_tensor("o_part", [TOK, DIM], F16, kind="ExternalOutput")

    with tile.TileContext(nc) as tc:
        with tc.tile_pool(name="persist", bufs=1) as persist, \
             tc.tile_pool(name="dram", bufs=1, space="DRAM") as dram, \
             tc.tile_pool(name="w1", bufs=1) as wpool, \
             tc.tile_pool(name="x1", bufs=2) as xpool, \
             tc.tile_pool(name="ev1", bufs=2) as evpool, \
             tc.tile_pool(name="at", bufs=1) as atpool, \
             tc.tile_pool(name="vbh", bufs=2) as vpool, \
             tc.tile_pool(name="e2", bufs=7) as epool, \
             tc.tile_pool(name="sm2", bufs=2) as smpool, \
             tc.tile_pool(name="st3", bufs=5) as stpool:
            qt = [persist.tile([P, TOK], F16, tag=f"qt{h}", name=f"qt{h}")
                  for h in range(2)]
            kt = [persist.tile([P, TOK], F16, tag=f"kt{h}", name=f"kt{h}")
                  for h in range(2)]
            ones_t = persist.tile([P, P], AVD, tag="ones")
            # batch-0 attention operands are prefetched mid-projection
            vbh0 = [persist.tile([P, NJC, P], AVD, tag=f"vbh0{h}",
                                 name=f"vbh0{h}")
                    for h in range(2)]
            wo_t = persist.tile([P, 2, DIM], F16, tag="wo")
            v_dram = dram.tile([TOK, INNER_C], AVD)
            at = [atpool.tile([P, TOK], F16, tag=f"at{h}", name=f"at{h}")
                  for h in range(2)]

            psA_ctx = tc.tile_pool(name="psA", bufs=1, space="PSUM")
            psA = psA_ctx.__enter__()

            # PE warm-up: matmuls on a zeroed tile, results discarded.
            # Starts the Tensor-engine p-state ramp while the first
            # weight/x DMAs are in flight.
            wz = evpool.tile([P, TC], F16, tag="warm_z")
            nc.vector.memzero(wz[:])
            wps = psA.tile([P, TC], F32, tag="ps_qk", bufs=2)
            for _ in range(N_WARM):
                nc.tensor.matmul(wps[:], wz[:, 0:P], wz[:],
                                 start=True, stop=True)

            wq_t = wpool.tile([P, KC, INNER_C], MM, tag="wq")
            wk_t = wpool.tile([P, KC, INNER_C], MM, tag="wk")
            wv_t = wpool.tile([P, KC, INNER_C], MM, tag="wv")
            # Two parallel DMA queues: SP streams weights (wq/wk
            # interleaved chunk-wise), ACT streams x chunks.
            nc.sync.dma_start(wq_t[:, 0:1, :], wq_d[:, 0:1, :])
            xt0 = xpool.tile([P, KC, TC], MM, tag="xt")
            nc.scalar.dma_start(xt0[:, 0:1, :], xT_d[0, :, 0:1, :])
            nc.sync.dma_start(wq_t[:, 1:4, :], wq_d[:, 1:4, :])
            nc.scalar.dma_start(xt0[:, 1:2, :], xT_d[0, :, 1:2, :])
            nc.sync.dma_start(wk_t[:, 0:4, :], wk_d[:, 0:4, :])
            nc.scalar.dma_start(xt0[:, 2:4, :], xT_d[0, :, 2:4, :])
            nc.sync.dma_start(wq_t[:, 4:KC, :], wq_d[:, 4:KC, :])
            nc.scalar.dma_start(xt0[:, 4:KC // 2, :],
                                xT_d[0, :, 4:KC // 2, :])
            nc.sync.dma_start(wk_t[:, 4:KC, :], wk_d[:, 4:KC, :])
            nc.scalar.dma_start(xt0[:, KC // 2:KC, :],
                                xT_d[0, :, KC // 2:KC, :])
            cos0 = evpool.tile([P, TC], F32, tag="cos")
            sin0 = evpool.tile([P, TC], F32, tag="sin")
            nc.sync.dma_start(wv_t[:, 0:KC // 2, :], wv_d[:, 0:KC // 2, :])
            nc.scalar.dma_start(cos0[:], cos_d[:, 0:TC])
            nc.scalar.dma_start(sin0[:], sin_d[:, 0:TC])
            nc.sync.dma_start(wv_t[:, KC // 2:KC, :], wv_d[:, KC // 2:KC, :])
            nc.scalar.dma_start(ones_t[:], ones_d[:])

            # ---------- attention micro-step machinery -----------------
            def attn_ic_gen(b, h, i0, iw, vbh):
                """Window-A generator: one j-chunk of attention per step
                (score matmul -> exp -> AV accumulate), then the softmax
                finish.  Yields between steps so the caller can interleave
                projection work into the PE stream."""
                boff = b * S
                isl = slice(boff + i0, boff + i0 + iw)
                ps_at = psA.tile([P, IC], F32, tag="ps_at", bufs=2)
                acc = smpool.tile([P, IC], AVD, tag="acc")
                acc2 = smpool.tile([P, IC], AVD, tag="acc2")
                es = [None] * NJC

                def sp_step(jc):
                    ps_s = psA.tile([P, IC], F32, tag="ps_s", bufs=2)
                    jsl = slice(boff + jc * P, boff + (jc + 1) * P)
                    nc.tensor.matmul(ps_s[:, 0:iw], kt[h][:, jsl],
                                     qt[h][:, isl], start=True, stop=True)
                    e = epool.tile([P, IC], AVD, tag="e")
                    nc.scalar.activation(
                        e[:, 0:iw], ps_s[:, 0:iw],
                        mybir.ActivationFunctionType.Exp, scale=SCALE)
                    es[jc] = e

                def a_step(jc):
                    e = es[jc]
                    nc.tensor.matmul(ps_at[:, 0:iw], vbh[:, jc, :],
                                     e[:, 0:iw],
                                     start=(jc == 0), stop=(jc == NJC - 1))
                    dst = acc if jc % 2 == 0 else acc2
                    if jc < 2:
                        nc.vector.tensor_copy(dst[:, 0:iw], e[:, 0:iw])
                    else:
                        nc.vector.tensor_add(dst[:, 0:iw], dst[:, 0:iw],
                                             e[:, 0:iw])

                # score-only lead: the first AV matmul waits for the vbh
                # gather, so give it a few extra steps of slack
                LEAD = 4
                for jc in range(LEAD):
                    sp_step(jc)
                    yield
                for jc in range(LEAD, NJC):
                    sp_step(jc)
                    a_step(jc - LEAD)
                    yield
                for jc in range(NJC - LEAD, NJC):
                    a_step(jc)
                nc.vector.tensor_add(acc[:, 0:iw], acc[:, 0:iw],
                                     acc2[:, 0:iw])
                yield
                ps_bc = psA.tile([P, IC], F32, tag="ps_bc", bufs=1)
                nc.tensor.matmul(ps_bc[:, 0:iw], ones_t[:], acc[:, 0:iw],
                                 start=True, stop=True)
                recip = smpool.tile([P, IC], F32, tag="recip")
                nc.vector.reciprocal_approx_fast(recip[:, 0:iw],
                                                 ps_bc[:, 0:iw])
                nc.vector.tensor_mul(at[h][:, isl], ps_at[:, 0:iw],
                                     recip[:, 0:iw])
                yield

            def chain(gens):
                for g in gens:
                    yield from g

            def drain(gen, n):
                for _ in range(n):
                    if next(gen, "done") == "done":
                        return False
                return True

            # ---------- window A: projections + batch-0 attention ------
            b0_steps = chain([attn_ic_gen(0, h, icn * IC, IC, vbh0[h])
                              for h in range(2) for icn in range(NIC)])

            for tcn in range(NTC):
                if tcn == 3:
                    # v_dram rows 0-1535 (tcn 0-2) are ready: start the
                    # strided vbh(0,0) gather early so the first AV matmul
                    # (early in tcn 4) does not wait on it
                    nc.sync.dma_start(
                        vbh0[0][:, 0:12, :],
                        v_dram[0:12 * P, 0:P]
                        .rearrange("(c p) d -> p c d", p=P))
                elif tcn == 4:
                    nc.sync.dma_start(
                        vbh0[0][:, 12:NJC, :],
                        v_dram[12 * P:S, 0:P]
                        .rearrange("(c p) d -> p c d", p=P))
                elif tcn == 5:
                    nc.sync.dma_start(
                        vbh0[1][:],
                        v_dram[0:S, P:2 * P]
                        .rearrange("(c p) d -> p c d", p=P))
                elif tcn == 6:
                    nc.sync.dma_start(wo_t[:], wo_d[:])
                elif tcn == 7:
                    # v rows 2048-3583 (tcn 4-6) are ready: start the
                    # batch-1 h0 vbh gather during the last token chunk
                    vbh10 = vpool.tile([P, NJC, P], AVD, tag="vbh",
                                       name="vbh10")
                    nc.sync.dma_start(
                        vbh10[:, 0:12, :],
                        v_dram[S:S + 12 * P, 0:P]
                        .rearrange("(c p) d -> p c d", p=P))
                tsl = slice(tcn * TC, (tcn + 1) * TC)
                if tcn == 0:
                    xt, cos_t, sin_t = xt0, cos0, sin0
                else:
                    xt = xpool.tile([P, KC, TC], MM, tag="xt")
                    nc.scalar.dma_start(xt[:], xT_d[tcn])
                    cos_t = evpool.tile([P, TC], F32, tag="cos")
                    sin_t = evpool.tile([P, TC], F32, tag="sin")
                    nc.sync.dma_start(cos_t[:], cos_d[:, tsl])
                    nc.sync.dma_start(sin_t[:], sin_d[:, tsl])

                # Q^T / K^T chunks with fused RoPE eviction.  For the
                # first token chunk the two head-halves are interleaved
                # kc-half-wise so the early matmuls consume DMA pieces as
                # they arrive instead of waiting for the chunk tail.
                for wt, dsts, nm in ((wq_t, qt, "q"), (wk_t, kt, "k")):
                    pss = [None, None]
                    if tcn == 0:
                        for m in range(2):
                            pss[m] = psA.tile([P, TC], F32, tag="ps_qk",
                                              bufs=2, name=f"ps0_{nm}{m}")
                        for kc_half in range(2):
                            for m in range(2):
                                for kc in range(kc_half * (KC // 2),
                                                (kc_half + 1) * (KC // 2)):
                                    nc.tensor.matmul(
                                        pss[m][:],
                                        wt[:, kc, m * P:(m + 1) * P],
                                        xt[:, kc, :],
                                        start=(kc == 0), stop=(kc == KC - 1))
                    for m in range(2):
                        if tcn == 0:
                            ps = pss[m]
                        else:
                            ps = psA.tile([P, TC], F32, tag="ps_qk", bufs=2)
                            for kc in range(KC):
                                nc.tensor.matmul(
                                    ps[:], wt[:, kc, m * P:(m + 1) * P],
                                    xt[:, kc, :],
                                    start=(kc == 0), stop=(kc == KC - 1))
                        tcos = evpool.tile([P, TC], F32, tag="tcos")
                        nc.vector.tensor_mul(tcos[:], ps[:], cos_t[:])
                        tsin = evpool.tile([P, TC], F32, tag="tsin")
                        nc.vector.scalar_tensor_tensor(
                            tsin[0:64, :], ps[64:128, :], 1.0,
                            sin_t[64:128, :], Mul, Mul)
                        nc.vector.scalar_tensor_tensor(
                            tsin[64:128, :], ps[0:64, :], 1.0,
                            sin_t[0:64, :], Mul, Mul)
                        nc.vector.tensor_add(dsts[m][:, tsl], tcos[:],
                                             tsin[:])
                        if tcn >= 4:
                            drain(b0_steps, 5)

                # V chunks (tokens on partitions) -> DRAM scratch
                for m in range(TC // P):
                    ps = psA.tile([P, INNER_C], F32, tag="ps_v", bufs=1)
                    for kc in range(KC):
                        nc.tensor.matmul(
                            ps[:], xt[:, kc, m * P:(m + 1) * P],
                            wv_t[:, kc, :],
                            start=(kc == 0), stop=(kc == KC - 1))
                    vst = evpool.tile([P, INNER_C], AVD, tag="vst")
                    nc.scalar.copy(vst[:], ps[:])
                    r0 = tcn * TC + m * P
                    nc.sync.dma_start(v_dram[r0:r0 + P, :], vst[:])
                    if tcn >= 4:
                        drain(b0_steps, 4)

            # flush any remaining batch-0 attention steps
            while drain(b0_steps, 8):
                pass

            psA_ctx.__exit__(None, None, None)

            # ---------- window B: batch-1 attention + out-projection ---
            # Fresh PSUM pool: wide score tiles (2 j-chunks per exp) fit
            # now that the projection banks are free.
            # Banks: ps_s 2x[P,1024]=4 + ps_at 2 + ps_o 2 = 8.
            psB_ctx = tc.tile_pool(name="psB", bufs=1, space="PSUM")
            psB = psB_ctx.__enter__()

            def load_vbh(b, h):
                boff = b * S
                vbh = vpool.tile([P, NJC, P], AVD, tag="vbh")
                nc.sync.dma_start(
                    vbh[:],
                    v_dram[boff:boff + S, h * P:(h + 1) * P]
                    .rearrange("(c p) d -> p c d", p=P))
                return vbh

            def attn_ic_gen_w(b, h, i0, iw, vbh):
                """Window-B generator: two j-chunks per step with a single
                wide exp (halves ACT instruction count).  The odd
                denominator-add chain runs on the otherwise idle GPSIMD."""
                boff = b * S
                isl = slice(boff + i0, boff + i0 + iw)
                ps_at = psB.tile([P, IC], F32, tag="ps_at", bufs=2)
                acc = smpool.tile([P, 2 * IC], AVD, tag="acc_w")
                acc2 = smpool.tile([P, 2 * IC], AVD, tag="acc2_w")
                NJP = NJC // 2
                es = [None] * NJP

                def sp_step(p):
                    ps_s = psB.tile([P, 2 * IC], F32, tag="ps_s", bufs=2)
                    for half in range(2):
                        jc = 2 * p + half
                        jsl = slice(boff + jc * P, boff + (jc + 1) * P)
                        nc.tensor.matmul(
                            ps_s[:, half * iw:(half + 1) * iw],
                            kt[h][:, jsl], qt[h][:, isl],
                            start=True, stop=True)
                    e = epool.tile([P, 2 * IC], AVD, tag="e_w")
                    nc.scalar.activation(
                        e[:, 0:2 * iw], ps_s[:, 0:2 * iw],
                        mybir.ActivationFunctionType.Exp, scale=SCALE)
                    es[p] = e

                def a_step(p):
                    e = es[p]
                    for half in range(2):
                        jc = 2 * p + half
                        nc.tensor.matmul(ps_at[:, 0:iw], vbh[:, jc, :],
                                         e[:, half * iw:(half + 1) * iw],
                                         start=(jc == 0),
                                         stop=(jc == NJC - 1))
                    # two independent accumulators halve the serial
                    # dependency chain of the denominator adds
                    dst = acc if p % 2 == 0 else acc2
                    if p < 2:
                        nc.vector.tensor_copy(dst[:, 0:2 * iw],
                                              e[:, 0:2 * iw])
                    else:
                        nc.vector.tensor_add(dst[:, 0:2 * iw],
                                             dst[:, 0:2 * iw],
                                             e[:, 0:2 * iw])

                sp_step(0)
                yield
                for p in range(1, NJP):
                    sp_step(p)
                    a_step(p - 1)
                    yield
                a_step(NJP - 1)
                yield
                # the ones-matmul accumulates both denominator
                # accumulators directly in PSUM (no DVE combine)
                ps_bc = psB.tile([P, IC], F32, tag="ps_o", bufs=2)
                nc.tensor.matmul(ps_bc[:, 0:iw], ones_t[:], acc[:, 0:iw],
                                 start=True, stop=False)
                nc.tensor.matmul(ps_bc[:, 0:iw], ones_t[:],
                                 acc[:, iw:2 * iw], start=False, stop=False)
                nc.tensor.matmul(ps_bc[:, 0:iw], ones_t[:], acc2[:, 0:iw],
                                 start=False, stop=False)
                nc.tensor.matmul(ps_bc[:, 0:iw], ones_t[:],
                                 acc2[:, iw:2 * iw], start=False, stop=True)
                recip = smpool.tile([P, IC], F32, tag="recip")
                nc.vector.reciprocal_approx_fast(recip[:, 0:iw],
                                                 ps_bc[:, 0:iw])
                nc.vector.tensor_mul(at[h][:, isl], ps_at[:, 0:iw],
                                     recip[:, 0:iw])
                yield

            def ph3_en(tn, en, stage, deep=False):
                """One 512-col en-group of the out-projection for token
                chunk tn; evictions split into 256-wide halves over
                ACT/DVE (~3.5 ACT / 4.5 DVE per tn on average).  In the
                final flush (deep=True) alternate en-groups borrow the
                idle ps_s banks so evictions stop gating the PSUM
                rotation."""
                if deep and en % 2 == 0:
                    ps = psB.tile([P, 2 * IC], F32, tag="ps_s", bufs=2)
                else:
                    ps = psB.tile([P, IC], F32, tag="ps_o", bufs=2)
                esl = slice(en * IC, (en + 1) * IC)
                for h in range(2):
                    nc.tensor.matmul(
                        ps[:, 0:IC], at[h][:, tn * P:(tn + 1) * P],
                        wo_t[:, h, esl],
                        start=(h == 0), stop=(h == 1))
                HB = IC // 2
                lo = slice(en * IC, en * IC + HB)
                hi = slice(en * IC + HB, (en + 1) * IC)
                act_lo = (en + tn) % 2 == 0
                if en == 3 and tn % 2 == 0:
                    nc.vector.tensor_copy(stage[:, lo], ps[:, 0:HB])
                    nc.vector.tensor_copy(stage[:, hi], ps[:, HB:IC])
                elif act_lo:
                    nc.scalar.copy(stage[:, lo], ps[:, 0:HB])
                    nc.vector.tensor_copy(stage[:, hi], ps[:, HB:IC])
                else:
                    nc.vector.tensor_copy(stage[:, lo], ps[:, 0:HB])
                    nc.scalar.copy(stage[:, hi], ps[:, HB:IC])
                if en % 2 == 1:
                    q = nc.sync if en == 1 else nc.scalar
                    q.dma_start(
                        o_d[tn * P:(tn + 1) * P,
                            (en - 1) * IC:(en + 1) * IC],
                        stage[:, (en - 1) * IC:(en + 1) * IC])

            class Ph3Queue:
                """Feeds out-projection en-groups one at a time."""
                def __init__(self):
                    self.items = []
                    self.stages = {}

                def push_tn(self, tn):
                    self.stages[tn] = stpool.tile([P, DIM], F16, tag="stage",
                                                  name=f"stage{tn}")
                    for en in range(DIM // IC):
                        self.items.append((tn, en))

                def step(self, n=1):
                    for _ in range(n):
                        if not self.items:
                            return
                        tn, en = self.items.pop(0)
                        ph3_en(tn, en, self.stages[tn])

                def flush(self):
                    while self.items:
                        tn, en = self.items.pop(0)
                        ph3_en(tn, en, self.stages[tn], deep=True)

            ph3 = Ph3Queue()
            for tn in range(16):        # batch-0 out-projection ready now
                ph3.push_tn(tn)

            def attn_with_ph3(b, h, i0, iw, vbh, rate=3):
                g = attn_ic_gen_w(b, h, i0, iw, vbh)
                k = 0
                while next(g, "done") != "done":
                    # rate/2 ph3 en-groups per step keeps PE dense
                    ph3.step(rate // 2 if k % 2 == 0 else rate - rate // 2)
                    k += 1

            nc.sync.dma_start(
                vbh10[:, 12:NJC, :],
                v_dram[S + 12 * P:2 * S, 0:P]
                .rearrange("(c p) d -> p c d", p=P))
            vbh = vbh10
            vbh11 = None
            for icn in range(NIC):
                if icn == NIC - 1:
                    vbh11 = load_vbh(1, 1)
                attn_with_ph3(1, 0, icn * IC, IC, vbh)
            vbh = vbh11
            for icn in range(NIC - 1):
                attn_with_ph3(1, 1, icn * IC, IC, vbh)
                for k in range(4):
                    ph3.push_tn(16 + icn * 4 + k)
            # the last i-chunk runs as two 256-query micro-chunks, each
            # unlocking two out-projection token groups early -- the
            # final flush shrinks to two tns
            for q in range(2):
                attn_with_ph3(1, 1, 3 * IC + q * 2 * P, 2 * P, vbh,
                              rate=2 if q == 0 else 1)
                ph3.push_tn(28 + 2 * q)
                ph3.push_tn(29 + 2 * q)
            ph3.flush()

            psB_ctx.__exit__(None, None, None)

    nc.finalize()
    return nc


def _rope_tables():
    """cos/sin tables in [head_dim, token] layout, matching the reference's
    f32 computation (jax on CPU when available)."""
    try:
        import jax
        import jax.numpy as jnp
        cpu = jax.devices("cpu")[0]
        with jax.default_device(cpu):
            inv = 1.0 / (10000.0 ** (
                jnp.arange(0, P, 2, dtype=jnp.float32) / P))
            t = jnp.arange(S, dtype=jnp.float32)
            freqs = jnp.einsum("i,j->ij", t, inv)          # [S, 64]
            emb = jnp.concatenate((freqs, freqs), axis=-1)  # [S, 128]
            cos = np.asarray(jnp.cos(emb)).T                # [128, S]
            sin = np.asarray(jnp.sin(emb)).T
    except Exception:
        inv = 1.0 / (10000.0 ** (np.arange(0, P, 2, dtype=np.float64) / P))
        t = np.arange(S, dtype=np.float64)
        freqs = np.outer(t, inv)
        emb = np.concatenate((freqs, freqs), axis=-1)
        cos = np.cos(emb).T.astype(np.float32)
        sin = np.sin(emb).T.astype(np.float32)

    cos2 = np.ascontiguousarray(np.tile(cos, (1, B)).astype(np.float32))
    sin_z = np.concatenate([sin[0:64], -sin[64:128]], axis=0)
    sin2 = np.ascontiguousarray(np.tile(sin_z, (1, B)).astype(np.float32))
    return cos2, sin2


_NC_CACHE = None


def _rearr_w(w):
    """[DIM, m] -> [P, KC, m] with partition-contiguous rows."""
    m = w.shape[1]
    return np.ascontiguousarray(
        w.reshape(KC, P, m).transpose(1, 0, 2)).astype(MM_NP)


def _in_maps(x, Wq, Wk, Wv, Wo):
    # [NTC, P, KC, TC]: xT[n, p, c, t] = x^T[c*P+p, n*TC+t] -- each token
    # chunk is one fully-contiguous 16KB-per-partition DMA
    xT = np.ascontiguousarray(
        x.reshape(TOK, DIM).T.reshape(KC, P, NTC, TC).transpose(2, 1, 0, 3)
    ).astype(MM_NP)
    cosT, sinZ = _rope_tables()
    ones = np.ones((P, P), dtype=AVD_NP)
    maps = []
    for c in range(N_CORES):
        cs = slice(c * INNER_C, (c + 1) * INNER_C)
        maps.append({
            "xT": xT,
            "wq": _rearr_w(Wq[:, cs]),
            "wk": _rearr_w(Wk[:, cs]),
            "wv": _rearr_w(Wv[:, cs]),
            "wo": np.ascontiguousarray(
                Wo[cs, :].reshape(2, P, DIM).transpose(1, 0, 2)
            ).astype(np.float16),
            "cosT": cosT,
            "sinZ": sinZ,
            "ones": ones,
        })
    return maps


def kernel(x, Wq, Wk, Wv, Wo):
    global _NC_CACHE
    assert x.shape == (B, S, DIM)
    if _NC_CACHE is None:
        _NC_CACHE = _build()
    in_maps = _in_maps(x, Wq, Wk, Wv, Wo)
    last_err = None
    for attempt in range(3):
        try:
            res = run_bass_kernel_spmd(_NC_CACHE, in_maps,
                                       core_ids=list(range(N_CORES)),
                                       trace=False)
            break
        except Exception as e:  # transient NRT faults: retry
            last_err = e
    else:
        raise last_err
    out = res.results[0]["o_part"].astype(np.float32)
    for c in range(1, N_CORES):
        out += res.results[c]["o_part"].astype(np.float32)
    return out.astype(np.float32).reshape(B, S, DIM)


# revision 49
# speedup vs baseline: 1.1972x; 1.1972x over previous
"""Trainium2 Bass kernel for nn_Attention_11287174054323.

Full attention layer: QKV projections + RoPE + softmax attention + output
projection.  B=2, S=2048, DIM=2048, 16 heads x 128 head_dim, fp32 I/O.

Sharding: tensor-parallel over heads across 8 NeuronCores (2 heads/core).
Each core computes q/k/v projections for its head slice, full attention for
its heads, and a partial output projection (row slice of Wo); the host sums
the 8 partials.

Structure (single merged schedule, one PE stream; measured 394.6us on HW
at the 2.37GHz Tensor p-state, ~466us when the device sits at ~1.98GHz):
  - Window A: QKV projections (token chunks tcn 0-7, fp16, RoPE fused into
    the Q/K PSUM eviction).  Batch-0 attention only needs tcn 0-3 outputs,
    so its micro-steps (one 128-key score matmul -> exp -> AV accumulate)
    are interleaved into the projection stream for tcn 4-7: the exp work
    lands on the otherwise-idle ACT engine while projections keep the PE
    dense, and projection matmuls fill the exp-latency gaps of attention.
    tcn 0 runs all four Q/K head-groups concurrently in kc-quarter
    round-robin so the startup is paced by DMA arrival, not chunk tails;
    PE warm-up matmuls at t=0 start the p-state ramp during the first
    DMAs.  vbh/wo for the attention windows prefetch mid-projection.
  - Window B (own PSUM pool): batch-1 attention with wide score steps
    (two j-chunks per exp, halving ACT instruction count) and the output
    projection (ph3) finely interleaved per 512-col en-group; the last
    i-chunk runs as two 256-query micro-chunks so the final out-projection
    groups unlock early, and the flush borrows the idle ps_s banks.
    PSUM evictions are split into 256-wide halves spread ~3.5/4.5 over
    ACT and DVE.
  - Softmax normalization is deferred past the AV accumulation; the
    denominator is accumulated with DVE adds in two independent
    accumulators whose partition-sums the ones-matmul accumulates
    directly in PSUM (no combine), inverted with reciprocal_approx_fast.
  - PSUM budget (8 banks), window A: qk(2) + v(1) + s(2) + at(2) + bc(1);
    window B: s(2x2) + at(2) + o/bc(2).
  - The partial output is staged to fp16 and summed on the host in fp32.
"""

import sys

sys.path.insert(0, "/opt/trn_rl_repo")

import numpy as np

import concourse.tile as tile
import concourse.mybir as mybir
from concourse import bacc
from concourse.bass_utils import run_bass_kernel_spmd

P = 128
B, S, DIM = 2, 2048, 2048
TOK = B * S                     # 4096 tokens
HEADS_PER_CORE = 2
INNER_C = HEADS_PER_CORE * P    # 256 per-core inner dim
KC = DIM // P                   # 16 contraction chunks
TC = 512                        # phase-1 token chunk
NTC = TOK // TC                 # 8
IC = 512                        # attention i-chunk (queries)
NIC = S // IC                   # 4 per (batch, head)
NJC = S // P                    # 16 key chunks per (batch, head)
SCALE = float(P) ** -0.5

F32 = mybir.dt.float32
F16 = mybir.dt.float16
MM = mybir.dt.float16           # matmul dtype for projections (x, wq/wk/wv)
MM_NP = np.float16
AVD = mybir.dt.float16          # dtype of the exp/AV group (e, acc, v, ones)
AVD_NP = np.float16

N_CORES = 8
Mul = mybir.AluOpType.mult
N_WARM = 18                     # PE warm-up matmuls


def _build():
    nc = bacc.Bacc("TRN2", target_bir_lowering=False)

    xT_d = nc.dram_tensor("xT", [NTC, P, KC, TC], MM, kind="ExternalInput")
    wq_d = nc.dram_tensor("wq", [P, KC, INNER_C], MM, kind="ExternalInput")
    wk_d = nc.dram_tensor("wk", [P, KC, INNER_C], MM, kind="ExternalInput")
    wv_d = nc.dram_tensor("wv", [P, KC, INNER_C], MM, kind="ExternalInput")
    wo_d = nc.dram_tensor("wo", [P, 2, DIM], F16, kind="ExternalInput")
    cos_d = nc.dram_tensor("cosT", [P, TOK], F32, kind="ExternalInput")
    sin_d = nc.dram_tensor("sinZ", [P, TOK], F32, kind="ExternalInput")
    ones_d = nc.dram_tensor("ones", [P, P], AVD, kind="ExternalInput")
    o_d = nc.dram_tensor("o_part", [TOK, DIM], F16, kind="ExternalOutput")

    with tile.TileContext(nc) as tc:
        with tc.tile_pool(name="persist", bufs=1) as persist, \
             tc.tile_pool(name="dram", bufs=1, space="DRAM") as dram, \
             tc.tile_pool(name="w1", bufs=1) as wpool, \
             tc.tile_pool(name="x1", bufs=2) as xpool, \
             tc.tile_pool(name="ev1", bufs=2) as evpool, \
             tc.tile_pool(name="at", bufs=1) as atpool, \
             tc.tile_pool(name="vbh", bufs=2) as vpool, \
             tc.tile_pool(name="e2", bufs=7) as epool, \
             tc.tile_pool(name="sm2", bufs=2) as smpool, \
             tc.tile_pool(name="st3", bufs=5) as stpool:
            qt = [persist.tile([P, TOK], F16, tag=f"qt{h}", name=f"qt{h}")
                  for h in range(2)]
            kt = [persist.tile([P, TOK], F16, tag=f"kt{h}", name=f"kt{h}")
                  for h in range(2)]
            ones_t = persist.tile([P, P], AVD, tag="ones")
            # batch-0 attention operands are prefetched mid-projection
            vbh0 = [persist.tile([P, NJC, P], AVD, tag=f"vbh0{h}",
                                 name=f"vbh0{h}")
                    for h in range(2)]
            wo_t = persist.tile([P, 2, DIM], F16, tag="wo")
            v_dram = dram.tile([TOK, INNER_C], AVD)
            at = [atpool.tile([P, TOK], F16, tag=f"at{h}", name=f"at{h}")
                  for h in range(2)]

            psA_ctx = tc.tile_pool(name="psA", bufs=1, space="PSUM")
            psA = psA_ctx.__enter__()

            # PE warm-up: matmuls on a zeroed tile, results discarded.
            # Starts the Tensor-engine p-state ramp while the first
            # weight/x DMAs are in flight.
            wz = evpool.tile([P, TC], F16, tag="warm_z")
            nc.vector.memzero(wz[:])
            wps = psA.tile([P, TC], F32, tag="ps_qk", bufs=2)
            for _ in range(N_WARM):
                nc.tensor.matmul(wps[:], wz[:, 0:P], wz[:],
                                 start=True, stop=True)

            wq_t = wpool.tile([P, KC, INNER_C], MM, tag="wq")
            wk_t = wpool.tile([P, KC, INNER_C], MM, tag="wk")
            wv_t = wpool.tile([P, KC, INNER_C], MM, tag="wv")
            # Two parallel DMA queues: SP streams weights (wq/wk
            # interleaved chunk-wise), ACT streams x chunks.
            nc.sync.dma_start(wq_t[:, 0:2, :], wq_d[:, 0:2, :])
            xt0 = xpool.tile([P, KC, TC], MM, tag="xt")
            nc.scalar.dma_start(xt0[:, 0:1, :], xT_d[0, :, 0:1, :])
            nc.sync.dma_start(wq_t[:, 2:4, :], wq_d[:, 2:4, :])
            nc.scalar.dma_start(xt0[:, 1:2, :], xT_d[0, :, 1:2, :])
            nc.sync.dma_start(wk_t[:, 0:4, :], wk_d[:, 0:4, :])
            nc.scalar.dma_start(xt0[:, 2:4, :], xT_d[0, :, 2:4, :])
            nc.sync.dma_start(wq_t[:, 4:8, :], wq_d[:, 4:8, :])
            nc.scalar.dma_start(xt0[:, 4:8, :], xT_d[0, :, 4:8, :])
            nc.sync.dma_start(wk_t[:, 4:8, :], wk_d[:, 4:8, :])
            nc.scalar.dma_start(xt0[:, 8:12, :], xT_d[0, :, 8:12, :])
            nc.sync.dma_start(wq_t[:, 8:KC, :], wq_d[:, 8:KC, :])
            cos0 = evpool.tile([P, TC], F32, tag="cos")
            sin0 = evpool.tile([P, TC], F32, tag="sin")
            nc.scalar.dma_start(cos0[:], cos_d[:, 0:TC])
            nc.scalar.dma_start(sin0[:], sin_d[:, 0:TC])
            nc.sync.dma_start(wk_t[:, 8:KC, :], wk_d[:, 8:KC, :])
            nc.scalar.dma_start(xt0[:, 12:KC, :], xT_d[0, :, 12:KC, :])
            nc.sync.dma_start(wv_t[:, 0:KC // 2, :], wv_d[:, 0:KC // 2, :])
            nc.sync.dma_start(wv_t[:, KC // 2:KC, :], wv_d[:, KC // 2:KC, :])
            nc.scalar.dma_start(ones_t[:], ones_d[:])

            # ---------- attention micro-step machinery -----------------
            def attn_ic_gen(b, h, i0, iw, vbh):
                """Window-A generator: one j-chunk of attention per step
                (score matmul -> exp -> AV accumulate), then the softmax
                finish.  Yields between steps so the caller can interleave
                projection work into the PE stream."""
                boff = b * S
                isl = slice(boff + i0, boff + i0 + iw)
                ps_at = psA.tile([P, IC], F32, tag="ps_at", bufs=2)
                acc = smpool.tile([P, IC], AVD, tag="acc")
                acc2 = smpool.tile([P, IC], AVD, tag="acc2")
                es = [None] * NJC

                def sp_step(jc):
                    ps_s = psA.tile([P, IC], F32, tag="ps_s", bufs=2)
                    jsl = slice(boff + jc * P, boff + (jc + 1) * P)
                    nc.tensor.matmul(ps_s[:, 0:iw], kt[h][:, jsl],
                                     qt[h][:, isl], start=True, stop=True)
                    e = epool.tile([P, IC], AVD, tag="e")
                    nc.scalar.activation(
                        e[:, 0:iw], ps_s[:, 0:iw],
                        mybir.ActivationFunctionType.Exp, scale=SCALE)
                    es[jc] = e

                def a_step(jc):
                    e = es[jc]
                    nc.tensor.matmul(ps_at[:, 0:iw], vbh[:, jc, :],
                                     e[:, 0:iw],
                                     start=(jc == 0), stop=(jc == NJC - 1))
                    dst = acc if jc % 2 == 0 else acc2
                    if jc < 2:
                        nc.vector.tensor_copy(dst[:, 0:iw], e[:, 0:iw])
                    else:
                        nc.vector.tensor_add(dst[:, 0:iw], dst[:, 0:iw],
                                             e[:, 0:iw])

                # score-only lead: the first AV matmul waits for the vbh
                # gather, so give it a few extra steps of slack
                LEAD = 4
                for jc in range(LEAD):
                    sp_step(jc)
                    yield
                for jc in range(LEAD, NJC):
                    sp_step(jc)
                    a_step(jc - LEAD)
                    yield
                for jc in range(NJC - LEAD, NJC):
                    a_step(jc)
                nc.vector.tensor_add(acc[:, 0:iw], acc[:, 0:iw],
                                     acc2[:, 0:iw])
                yield
                ps_bc = psA.tile([P, IC], F32, tag="ps_bc", bufs=1)
                nc.tensor.matmul(ps_bc[:, 0:iw], ones_t[:], acc[:, 0:iw],
                                 start=True, stop=True)
                recip = smpool.tile([P, IC], F32, tag="recip")
                nc.vector.reciprocal_approx_fast(recip[:, 0:iw],
                                                 ps_bc[:, 0:iw])
                nc.vector.tensor_mul(at[h][:, isl], ps_at[:, 0:iw],
                                     recip[:, 0:iw])
                yield

            def chain(gens):
                for g in gens:
                    yield from g

            def drain(gen, n):
                for _ in range(n):
                    if next(gen, "done") == "done":
                        return False
                return True

            # ---------- window A: projections + batch-0 attention ------
            b0_steps = chain([attn_ic_gen(0, h, icn * IC, IC, vbh0[h])
                              for h in range(2) for icn in range(NIC)])

            for tcn in range(NTC):
                if tcn == 3:
                    # v_dram rows 0-1535 (tcn 0-2) are ready: start the
                    # strided vbh(0,0) gather early so the first AV matmul
                    # (early in tcn 4) does not wait on it
                    nc.sync.dma_start(
                        vbh0[0][:, 0:12, :],
                        v_dram[0:12 * P, 0:P]
                        .rearrange("(c p) d -> p c d", p=P))
                elif tcn == 4:
                    nc.sync.dma_start(
                        vbh0[0][:, 12:NJC, :],
                        v_dram[12 * P:S, 0:P]
                        .rearrange("(c p) d -> p c d", p=P))
                elif tcn == 5:
                    nc.sync.dma_start(
                        vbh0[1][:],
                        v_dram[0:S, P:2 * P]
                        .rearrange("(c p) d -> p c d", p=P))
                elif tcn == 6:
                    nc.sync.dma_start(wo_t[:], wo_d[:])
                elif tcn == 7:
                    # v rows 2048-3583 (tcn 4-6) are ready: start the
                    # batch-1 h0 vbh gather during the last token chunk
                    vbh10 = vpool.tile([P, NJC, P], AVD, tag="vbh",
                                       name="vbh10")
                    nc.sync.dma_start(
                        vbh10[:, 0:12, :],
                        v_dram[S:S + 12 * P, 0:P]
                        .rearrange("(c p) d -> p c d", p=P))
                tsl = slice(tcn * TC, (tcn + 1) * TC)
                if tcn == 0:
                    xt, cos_t, sin_t = xt0, cos0, sin0
                else:
                    xt = xpool.tile([P, KC, TC], MM, tag="xt")
                    nc.scalar.dma_start(xt[:], xT_d[tcn])
                    cos_t = evpool.tile([P, TC], F32, tag="cos")
                    sin_t = evpool.tile([P, TC], F32, tag="sin")
                    nc.sync.dma_start(cos_t[:], cos_d[:, tsl])
                    nc.sync.dma_start(sin_t[:], sin_d[:, tsl])

                def rope_evict(ps, dsts, m):
                    tcos = evpool.tile([P, TC], F32, tag="tcos")
                    nc.vector.tensor_mul(tcos[:], ps[:], cos_t[:])
                    tsin = evpool.tile([P, TC], F32, tag="tsin")
                    nc.vector.scalar_tensor_tensor(
                        tsin[0:64, :], ps[64:128, :], 1.0,
                        sin_t[64:128, :], Mul, Mul)
                    nc.vector.scalar_tensor_tensor(
                        tsin[64:128, :], ps[0:64, :], 1.0,
                        sin_t[0:64, :], Mul, Mul)
                    nc.vector.tensor_add(dsts[m][:, tsl], tcos[:],
                                         tsin[:])

                def qk_groups():
                    """Q^T / K^T chunks with fused RoPE eviction.  For the
                    first token chunk all four head-groups run
                    concurrently (borrowing the not-yet-used ps_s banks),
                    interleaved kc-quarter-wise, so the early matmuls
                    consume DMA pieces as they arrive."""
                    if tcn == 0:
                        pss = {}
                        for wi, (wt, nm) in enumerate(((wq_t, "q"),
                                                       (wk_t, "k"))):
                            for m in range(2):
                                tag = "ps_qk" if wi == 0 else "ps_s"
                                pss[(wi, m)] = psA.tile(
                                    [P, TC], F32, tag=tag, bufs=2,
                                    name=f"ps0_{nm}{m}")
                        for kq in range(4):
                            for wi, (wt, nm) in enumerate(((wq_t, "q"),
                                                           (wk_t, "k"))):
                                for m in range(2):
                                    for kc in range(kq * 4, kq * 4 + 4):
                                        nc.tensor.matmul(
                                            pss[(wi, m)][:],
                                            wt[:, kc, m * P:(m + 1) * P],
                                            xt[:, kc, :],
                                            start=(kc == 0),
                                            stop=(kc == KC - 1))
                        for wi, (wt, dsts) in enumerate(((wq_t, qt),
                                                         (wk_t, kt))):
                            for m in range(2):
                                rope_evict(pss[(wi, m)], dsts, m)
                        return
                    for wt, dsts in ((wq_t, qt), (wk_t, kt)):
                        for m in range(2):
                            ps = psA.tile([P, TC], F32, tag="ps_qk", bufs=2)
                            for kc in range(KC):
                                nc.tensor.matmul(
                                    ps[:], wt[:, kc, m * P:(m + 1) * P],
                                    xt[:, kc, :],
                                    start=(kc == 0), stop=(kc == KC - 1))
                            rope_evict(ps, dsts, m)
                            if tcn >= 4:
                                drain(b0_steps, 5 if tcn < 6 else 6)

                def v_groups():
                    # V chunks (tokens on partitions) -> DRAM scratch
                    for m in range(TC // P):
                        ps = psA.tile([P, INNER_C], F32, tag="ps_v", bufs=1)
                        for kc in range(KC):
                            nc.tensor.matmul(
                                ps[:], xt[:, kc, m * P:(m + 1) * P],
                                wv_t[:, kc, :],
                                start=(kc == 0), stop=(kc == KC - 1))
                        vst = evpool.tile([P, INNER_C], AVD, tag="vst")
                        nc.scalar.copy(vst[:], ps[:])
                        r0 = tcn * TC + m * P
                        nc.sync.dma_start(v_dram[r0:r0 + P, :], vst[:])
                        if tcn >= 4:
                            drain(b0_steps, 4 if tcn < 6 else 5)

                qk_groups()
                v_groups()

            # flush any remaining batch-0 attention steps
            while drain(b0_steps, 8):
                pass

            psA_ctx.__exit__(None, None, None)

            # ---------- window B: batch-1 attention + out-projection ---
            # Fresh PSUM pool: wide score tiles (2 j-chunks per exp) fit
            # now that the projection banks are free.
            # Banks: ps_s 2x[P,1024]=4 + ps_at 2 + ps_o 2 = 8.
            psB_ctx = tc.tile_pool(name="psB", bufs=1, space="PSUM")
            psB = psB_ctx.__enter__()

            def load_vbh(b, h):
                boff = b * S
                vbh = vpool.tile([P, NJC, P], AVD, tag="vbh")
                nc.sync.dma_start(
                    vbh[:],
                    v_dram[boff:boff + S, h * P:(h + 1) * P]
                    .rearrange("(c p) d -> p c d", p=P))
                return vbh

            def attn_ic_gen_w(b, h, i0, iw, vbh):
                """Window-B generator: two j-chunks per step with a single
                wide exp (halves ACT instruction count).  The odd
                denominator-add chain runs on the otherwise idle GPSIMD."""
                boff = b * S
                isl = slice(boff + i0, boff + i0 + iw)
                ps_at = psB.tile([P, IC], F32, tag="ps_at", bufs=2)
                acc = smpool.tile([P, 2 * IC], AVD, tag="acc_w")
                acc2 = smpool.tile([P, 2 * IC], AVD, tag="acc2_w")
                NJP = NJC // 2
                es = [None] * NJP

                def sp_step(p):
                    ps_s = psB.tile([P, 2 * IC], F32, tag="ps_s", bufs=2)
                    for half in range(2):
                        jc = 2 * p + half
                        jsl = slice(boff + jc * P, boff + (jc + 1) * P)
                        nc.tensor.matmul(
                            ps_s[:, half * iw:(half + 1) * iw],
                            kt[h][:, jsl], qt[h][:, isl],
                            start=True, stop=True)
                    e = epool.tile([P, 2 * IC], AVD, tag="e_w")
                    nc.scalar.activation(
                        e[:, 0:2 * iw], ps_s[:, 0:2 * iw],
                        mybir.ActivationFunctionType.Exp, scale=SCALE)
                    es[p] = e

                def a_step(p):
                    e = es[p]
                    for half in range(2):
                        jc = 2 * p + half
                        nc.tensor.matmul(ps_at[:, 0:iw], vbh[:, jc, :],
                                         e[:, half * iw:(half + 1) * iw],
                                         start=(jc == 0),
                                         stop=(jc == NJC - 1))
                    # two independent accumulators halve the serial
                    # dependency chain of the denominator adds
                    dst = acc if p % 2 == 0 else acc2
                    if p < 2:
                        nc.vector.tensor_copy(dst[:, 0:2 * iw],
                                              e[:, 0:2 * iw])
                    else:
                        nc.vector.tensor_add(dst[:, 0:2 * iw],
                                             dst[:, 0:2 * iw],
                                             e[:, 0:2 * iw])

                sp_step(0)
                yield
                for p in range(1, NJP):
                    sp_step(p)
                    a_step(p - 1)
                    yield
                # the ones-matmul accumulates both denominator
                # accumulators directly in PSUM (no DVE combine); acc is
                # complete after a_step(NJP-2), so its two matmuls issue
                # before the final AV step to shorten the boundary chain
                ps_bc = psB.tile([P, IC], F32, tag="ps_o", bufs=2)
                nc.tensor.matmul(ps_bc[:, 0:iw], ones_t[:], acc[:, 0:iw],
                                 start=True, stop=False)
                nc.tensor.matmul(ps_bc[:, 0:iw], ones_t[:],
                                 acc[:, iw:2 * iw], start=False, stop=False)
                a_step(NJP - 1)
                yield
                nc.tensor.matmul(ps_bc[:, 0:iw], ones_t[:], acc2[:, 0:iw],
                                 start=False, stop=False)
                nc.tensor.matmul(ps_bc[:, 0:iw], ones_t[:],
                                 acc2[:, iw:2 * iw], start=False, stop=True)
                recip = smpool.tile([P, IC], F32, tag="recip")
                nc.vector.reciprocal_approx_fast(recip[:, 0:iw],
                                                 ps_bc[:, 0:iw])
                nc.vector.tensor_mul(at[h][:, isl], ps_at[:, 0:iw],
                                     recip[:, 0:iw])
                # no trailing yield: the next ph3 drain happens only after
                # the next ic's first score step, giving the reciprocal a
                # full step of slack before the next ps_o allocation

            def ph3_en(tn, en, stage, deep=False):
                """One 512-col en-group of the out-projection for token
                chunk tn; evictions split into 256-wide halves over
                ACT/DVE (~3.5 ACT / 4.5 DVE per tn on average).  In the
                final flush (deep=True) alternate en-groups borrow the
                idle ps_s banks so evictions stop gating the PSUM
                rotation."""
                if deep and en % 2 == 0:
                    ps = psB.tile([P, 2 * IC], F32, tag="ps_s", bufs=2)
                else:
                    ps = psB.tile([P, IC], F32, tag="ps_o", bufs=2)
                esl = slice(en * IC, (en + 1) * IC)
                for h in range(2):
                    nc.tensor.matmul(
                        ps[:, 0:IC], at[h][:, tn * P:(tn + 1) * P],
                        wo_t[:, h, esl],
                        start=(h == 0), stop=(h == 1))
                HB = IC // 2
                lo = slice(en * IC, en * IC + HB)
                hi = slice(en * IC + HB, (en + 1) * IC)
                act_lo = (en + tn) % 2 == 0
                if en == 3 and tn % 2 == 0:
                    nc.vector.tensor_copy(stage[:, lo], ps[:, 0:HB])
                    nc.vector.tensor_copy(stage[:, hi], ps[:, HB:IC])
                elif act_lo:
                    nc.scalar.copy(stage[:, lo], ps[:, 0:HB])
                    nc.vector.tensor_copy(stage[:, hi], ps[:, HB:IC])
                else:
                    nc.vector.tensor_copy(stage[:, lo], ps[:, 0:HB])
                    nc.scalar.copy(stage[:, hi], ps[:, HB:IC])
                if en % 2 == 1:
                    q = nc.sync if en == 1 else nc.scalar
                    q.dma_start(
                        o_d[tn * P:(tn + 1) * P,
                            (en - 1) * IC:(en + 1) * IC],
                        stage[:, (en - 1) * IC:(en + 1) * IC])

            class Ph3Queue:
                """Feeds out-projection en-groups one at a time."""
                def __init__(self):
                    self.items = []
                    self.stages = {}

                def push_tn(self, tn):
                    self.stages[tn] = stpool.tile([P, DIM], F16, tag="stage",
                                                  name=f"stage{tn}")
                    for en in range(DIM // IC):
                        self.items.append((tn, en))

                def step(self, n=1):
                    for _ in range(n):
                        if not self.items:
                            return
                        tn, en = self.items.pop(0)
                        ph3_en(tn, en, self.stages[tn])

                def flush(self):
                    while self.items:
                        tn, en = self.items.pop(0)
                        ph3_en(tn, en, self.stages[tn], deep=True)

            ph3 = Ph3Queue()
            for tn in range(16):        # batch-0 out-projection ready now
                ph3.push_tn(tn)

            def attn_with_ph3(b, h, i0, iw, vbh, rate=3):
                g = attn_ic_gen_w(b, h, i0, iw, vbh)
                k = 0
                while next(g, "done") != "done":
                    # rate/2 ph3 en-groups per step keeps PE dense
                    ph3.step(rate // 2 if k % 2 == 0 else rate - rate // 2)
                    k += 1

            nc.sync.dma_start(
                vbh10[:, 12:NJC, :],
                v_dram[S + 12 * P:2 * S, 0:P]
                .rearrange("(c p) d -> p c d", p=P))
            vbh = vbh10
            vbh11 = None
            for icn in range(NIC):
                if icn == NIC - 1:
                    vbh11 = load_vbh(1, 1)
                attn_with_ph3(1, 0, icn * IC, IC, vbh)
            vbh = vbh11
            for icn in range(NIC - 1):
                attn_with_ph3(1, 1, icn * IC, IC, vbh)
                for k in range(4):
                    ph3.push_tn(16 + icn * 4 + k)
            # the last i-chunk runs as two 256-query micro-chunks, each
            # unlocking two out-projection token groups early -- the
            # final flush shrinks to two tns
            for q in range(2):
                attn_with_ph3(1, 1, 3 * IC + q * 2 * P, 2 * P, vbh,
                              rate=2 if q == 0 else 1)
                ph3.push_tn(28 + 2 * q)
                ph3.push_tn(29 + 2 * q)
            ph3.flush()

            psB_ctx.__exit__(None, None, None)

    nc.finalize()
    return nc


def _rope_tables():
    """cos/sin tables in [head_dim, token] layout, matching the reference's
    f32 computation (jax on CPU when available)."""
    try:
        import jax
        import jax.numpy as jnp
        cpu = jax.devices("cpu")[0]
        with jax.default_device(cpu):
            inv = 1.0 / (10000.0 ** (
                jnp.arange(0, P, 2, dtype=jnp.float32) / P))
            t = jnp.arange(S, dtype=jnp.float32)
            freqs = jnp.einsum("i,j->ij", t, inv)          # [S, 64]
            emb = jnp.concatenate((freqs, freqs), axis=-1)  # [S, 128]
            cos = np.asarray(jnp.cos(emb)).T                # [128, S]
            sin = np.asarray(jnp.sin(emb)).T
    except Exception:
        inv = 1.0 / (10000.0 ** (np.arange(0, P, 2, dtype=np.float64) / P))
        t = np.arange(S, dtype=np.float64)
        freqs = np.outer(t, inv)
        emb = np.concatenate((freqs, freqs), axis=-1)
        cos = np.cos(emb).T.astype(np.float32)
        sin = np.sin(emb).T.astype(np.float32)

    cos2 = np.ascontiguousarray(np.tile(cos, (1, B)).astype(np.float32))
    sin_z = np.concatenate([sin[0:64], -sin[64:128]], axis=0)
    sin2 = np.ascontiguousarray(np.tile(sin_z, (1, B)).astype(np.float32))
    return cos2, sin2


_NC_CACHE = None


def _rearr_w(w):
    """[DIM, m] -> [P, KC, m] with partition-contiguous rows."""
    m = w.shape[1]
    return np.ascontiguousarray(
        w.reshape(KC, P, m).transpose(1, 0, 2)).astype(MM_NP)


def _in_maps(x, Wq, Wk, Wv, Wo):
    # [NTC, P, KC, TC]: xT[n, p, c, t] = x^T[c*P+p, n*TC+t] -- each token
    # chunk is one fully-contiguous 16KB-per-partition DMA
    xT = np.ascontiguousarray(
        x.reshape(TOK, DIM).T.reshape(KC, P, NTC, TC).transpose(2, 1, 0, 3)
    ).astype(MM_NP)
    cosT, sinZ = _rope_tables()
    ones = np.ones((P, P), dtype=AVD_NP)
    maps = []
    for c in range(N_CORES):
        cs = slice(c * INNER_C, (c + 1) * INNER_C)
        maps.append({
            "xT": xT,
            "wq": _rearr_w(Wq[:, cs]),
            "wk": _rearr_w(Wk[:, cs]),
            "wv": _rearr_w(Wv[:, cs]),
            "wo": np.ascontiguousarray(
                Wo[cs, :].reshape(2, P, DIM).transpose(1, 0, 2)
            ).astype(np.float16),
            "cosT": cosT,
            "sinZ": sinZ,
            "ones": ones,
        })
    return maps


def kernel(x, Wq, Wk, Wv, Wo):
    global _NC_CACHE
    assert x.shape == (B, S, DIM)
    if _NC_CACHE is None:
        _NC_CACHE = _build()
    in_maps = _in_maps(x, Wq, Wk, Wv, Wo)
    last_err = None
    for attempt in range(3):
        try:
            res = run_bass_kernel_spmd(_NC_CACHE, in_maps,
                                       core_ids=list(range(N_CORES)),
                                       trace=False)
            break
        except Exception as e:  # transient NRT faults: retry
            last_err = e
    else:
        raise last_err
    out = res.results[0]["o_part"].astype(np.float32)
    for c in range(1, N_CORES):
        out += res.results[c]["o_part"].astype(np.float32)
    return out.astype(np.float32).reshape(B, S, DIM)
